# revision 2
# baseline (speedup 1.0000x reference)
"""Trainium2 Bass kernel for a 2-layer GCN + CORAL head (nn_CORALClassifier).

Strategy (8 NeuronCores, SPMD single program):
  - Nodes sharded by destination across 8 cores (12500 nodes each, padded to
    12544 = 98 tiles of 128).
  - Node-feature table (g = h * deg_inv_sqrt, fp16) lives in DRAM, replicated
    via AllGather after each layer's shard-local compute.
  - Per GCN layer, each core gathers its in-edges' source rows with
    gpsimd.dma_gather (int16 indices -> 4 banks of 25088 rows), then reduces
    per 128-node destination tile on the PE with is_equal one-hot matmuls
    accumulated in PSUM, applies W + relu (scale fused in ACT), and
    AllGathers the new table.
  - CORAL head: shared logit = h2 @ w_fc via a fused multiply+row-reduce on
    the vector engine; the tiny threshold-bias broadcast happens on host.
"""

import sys

for _p in ("/root/.axon_site", "/root/.axon_site/_ro/trn_rl_repo",
           "/root/.axon_site/_ro/pypackages", "/opt/trn_rl_repo"):
    if _p not in sys.path:
        sys.path.append(_p)

import numpy as np

import concourse.bacc as bacc
import concourse.bass as bass
import concourse.tile as tile
from concourse import mybir
from concourse.bass_utils import run_bass_kernel_spmd

# ---------------------------------------------------------------- constants
N_NODES = 100000
N_FEAT = 128
HIDDEN = 128
NUM_CLASSES = 5
CORES = 8
SHARD = N_NODES // CORES            # 12500
TILES = 98                           # ceil(12500 / 128)
SHARD_PAD = TILES * 128              # 12544
NT = SHARD_PAD * CORES               # 100352 table rows
BANKS = 4
# Unequal banks (each < 32768 for int16 gather indices): a smaller last bank
# lets its column quota round down, saving a padded chunk per tile.
BANK_LO = [0, 26624, 53248, 79872, NT]
TPB = 7                              # tiles per gather block
NBLOCKS = TILES // TPB               # 14
P = 128
F16 = mybir.dt.float16
F32 = mybir.dt.float32
I16 = mybir.dt.int16

_CACHE = {}


# ------------------------------------------------------------ host-side prep
def _prep_edges(edge_index):
    """Build per-core dma_gather index arrays and one-hot dst-slot arrays.

    Layout per core, per layer-independent (same graph both layers):
      slots are grouped per (tile, bank) cell, each cell padded to m_b*128
      slots; cells laid out block-major: block B holds tiles [7B, 7B+7);
      within a block, bank-major: bank b occupies columns
      [7*sum(m[:b]), ...) with tile t's cell at column offset t*m_b.
    """
    src = edge_index[0].astype(np.int64)
    dst = edge_index[1].astype(np.int64)
    # self loops handled separately on-device (identity matmul per tile)

    core_e = dst // SHARD
    rem = dst % SHARD
    tile_e = rem // P
    slot_e = (rem % P).astype(np.float32)          # dst slot within tile
    pid_src = (src // SHARD) * SHARD_PAD + (src % SHARD)
    blo = np.asarray(BANK_LO, np.int64)
    bank_e = np.searchsorted(blo, pid_src, side="right") - 1
    bidx_e = (pid_src - blo[bank_e]).astype(np.int16)

    ncells = CORES * TILES * BANKS
    cellkey = ((core_e * TILES + tile_e) * BANKS + bank_e).astype(np.int64)
    counts = np.bincount(cellkey, minlength=ncells)
    percell = counts.reshape(CORES, TILES, BANKS)
    mb = np.maximum(1, np.ceil(percell.max(axis=(0, 1)) / P).astype(np.int64))
    K = int(mb.sum())                 # chunks (columns) per tile
    colspb = TPB * K                  # columns per block
    spb = colspb * P                  # slots per block
    spc = NBLOCKS * spb               # slots per core

    mcum = np.concatenate([[0], np.cumsum(mb)])
    # start slot of each cell within its core
    t_all = np.arange(TILES)
    blk = t_all // TPB
    tau = t_all % TPB
    # cellstart[t, b] relative to core start
    cellstart = (blk[:, None] * spb
                 + (mcum[None, :BANKS] * TPB + tau[:, None] * mb[None, :]) * P)
    cellstart_full = (np.arange(CORES)[:, None, None] * spc
                      + cellstart[None, :, :])          # [CORES, TILES, BANKS]

    order = np.argsort(cellkey, kind="stable")
    cum = np.concatenate([[0], np.cumsum(counts)])
    rank = np.empty_like(order)
    rank[order] = np.arange(len(order)) - cum[cellkey[order]]

    pos = cellstart_full.reshape(-1)[cellkey] + rank

    idxflat = np.zeros(CORES * spc, np.int16)       # pad -> bank row 0
    dstflat = np.full(CORES * spc, -1.0, np.float32)
    idxflat[pos] = bidx_e
    dstflat[pos] = slot_e

    # dstloc [CORES, NBLOCKS, 128, colspb]: slot s -> (col=s//128, p=s%128)
    dstloc = (dstflat.reshape(CORES, NBLOCKS, colspb, P)
              .transpose(0, 1, 3, 2).copy())
    # idx16 wrapped per (block, bank) call: call list i -> [i%16, i//16],
    # replicated over the 8 16-partition groups
    ni_b = (TPB * mb * P).astype(np.int64)          # num_idxs per call
    tot16 = int(ni_b.sum() // 16)
    idx16 = np.zeros((CORES, NBLOCKS, P, tot16), np.int16)
    idxv = idxflat.reshape(CORES, NBLOCKS, colspb * P)
    off = 0
    call_off16 = []
    for b in range(BANKS):
        n = int(ni_b[b])
        seg = idxv[:, :, off:off + n]                     # [CORES, NB, n]
        w = seg.reshape(CORES, NBLOCKS, n // 16, 16).transpose(0, 1, 3, 2)
        idx16[:, :, :, off // 16:(off + n) // 16] = np.tile(w, (1, 1, 8, 1))
        call_off16.append(off // 16)
        off += n
    return dict(idx16=idx16, dstloc=dstloc, mb=mb, K=K, colspb=colspb,
                ni_b=ni_b, call_off16=call_off16, mcum=mcum)


def _build_program(K, colspb, ni_b, call_off16, mb, with_bias, variant="full"):
    skip = set(variant.split("+")) if variant != "full" else set()
    nc = bacc.Bacc("TRN2", target_bir_lowering=False, debug=False,
                   num_devices=CORES)
    tot16 = int(np.sum(ni_b) // 16)

    # -------- per-core inputs
    t_x = nc.dram_tensor("xs", [SHARD_PAD, N_FEAT], F32, kind="ExternalInput")
    t_idx = nc.dram_tensor("idx16", [NBLOCKS, P, tot16], I16,
                           kind="ExternalInput")
    t_dst = nc.dram_tensor("dstloc", [NBLOCKS, P, colspb], F32,
                           kind="ExternalInput")
    t_disT = nc.dram_tensor("disT", [P, TILES], F32, kind="ExternalInput")
    t_dsqT = nc.dram_tensor("dissqT", [P, TILES], F32, kind="ExternalInput")
    t_w1 = nc.dram_tensor("w1h", [N_FEAT, HIDDEN], F16, kind="ExternalInput")
    t_w2 = nc.dram_tensor("w2h", [HIDDEN, HIDDEN], F16, kind="ExternalInput")
    t_wfc = nc.dram_tensor("wfcb", [P, HIDDEN], F16, kind="ExternalInput")
    t_iota = nc.dram_tensor("iota", [P, P], F16, kind="ExternalInput")
    t_ident = nc.dram_tensor("ident", [P, P], F16, kind="ExternalInput")
    t_out = nc.dram_tensor("outs", [P, TILES], F32, kind="ExternalOutput")

    # -------- internal DRAM
    ag0_in = nc.dram_tensor("ag0_in", [SHARD_PAD, N_FEAT], F16, kind="Internal")
    ag1_in = nc.dram_tensor("ag1_in", [SHARD_PAD, N_FEAT], F16, kind="Internal")
    # NOTE: dma_gather cannot read Shared-address-space DRAM (device crash),
    # so the AllGather outputs live in Local scratchpad.
    g0_full = nc.dram_tensor("g0_full", [NT, N_FEAT], F16, kind="Internal")
    g1_full = nc.dram_tensor("g1_full", [NT, N_FEAT], F16, kind="Internal")

    rg = [list(range(CORES))]

    with tile.TileContext(nc) as tc:
        with tc.tile_pool(name="const", bufs=1) as cpool, \
             tc.tile_pool(name="gath", bufs=2) as gpool, \
             tc.tile_pool(name="meta", bufs=2) as mpool, \
             tc.tile_pool(name="work", bufs=4) as wpool, \
             tc.tile_pool(name="outp", bufs=3) as opool, \
             tc.tile_pool(name="psA", bufs=4, space="PSUM") as psA, \
             tc.tile_pool(name="psH", bufs=2, space="PSUM") as psH:

            iota_t = cpool.tile([P, P], F16)
            nc.sync.dma_start(iota_t[:], t_iota[:])
            ident_t = cpool.tile([P, P], F16)
            nc.sync.dma_start(ident_t[:], t_ident[:])
            w1_t = cpool.tile([N_FEAT, HIDDEN], F16)
            nc.sync.dma_start(w1_t[:], t_w1[:])
            w2_t = cpool.tile([HIDDEN, HIDDEN], F16)
            nc.sync.dma_start(w2_t[:], t_w2[:])
            wfc_t = cpool.tile([P, HIDDEN], F16)
            nc.sync.dma_start(wfc_t[:], t_wfc[:])
            disT_t = cpool.tile([P, TILES], F32)
            nc.sync.dma_start(disT_t[:], t_disT[:])
            dsqT_t = cpool.tile([P, TILES], F32)
            nc.sync.dma_start(dsqT_t[:], t_dsqT[:])

            # ---- stage 0: g0 = x * dis (fp16), shard-local then AllGather
            for t in range(TILES):
                xin = wpool.tile([P, N_FEAT], F32, tag="xin")
                nc.sync.dma_start(xin[:], t_x[t * P:(t + 1) * P, :])
                g0t = opool.tile([P, N_FEAT], F16, tag="gout")
                nc.scalar.activation(g0t[:], xin[:],
                                     mybir.ActivationFunctionType.Copy,
                                     scale=disT_t[:, t:t + 1])
                nc.sync.dma_start(ag0_in[t * P:(t + 1) * P, :], g0t[:])
            if "no_ag" not in skip:
                nc.gpsimd.collective_compute(
                    "AllGather", mybir.AluOpType.bypass, replica_groups=rg,
                    ins=[ag0_in[:]], outs=[g0_full[:]])

            # ---- GCN layers
            shared_t = cpool.tile([P, TILES], F32)   # head accumulator cols
            for layer in range(2):
                table = g0_full if layer == 0 else g1_full
                own = ag0_in if layer == 0 else ag1_in
                w_t = w1_t if layer == 0 else w2_t
                for B in range(NBLOCKS):
                    idx_t = mpool.tile([P, tot16], I16, tag="idx")
                    nc.sync.dma_start(idx_t[:], t_idx[B])
                    dst_t = mpool.tile([P, colspb], F32, tag="dst")
                    nc.sync.dma_start(dst_t[:], t_dst[B])
                    if "no_gather" not in skip:
                        gath = gpool.tile([P, colspb * N_FEAT], F16, tag="gath")
                    else:
                        gath = None
                    colbase = 0
                    for b in range(BANKS if "no_gather" not in skip else 0):
                        ncols = TPB * int(mb[b])
                        nc.gpsimd.dma_gather(
                            out_ap=gath[:, colbase * N_FEAT:
                                        (colbase + ncols) * N_FEAT]
                            .rearrange("p (c f) -> p c f", c=ncols),
                            in_ap=table[BANK_LO[b]:BANK_LO[b + 1], :],
                            idxs_ap=idx_t[:, call_off16[b]:
                                          call_off16[b] + int(ni_b[b]) // 16],
                            num_idxs=int(ni_b[b]),
                            num_idxs_reg=int(ni_b[b]),
                            elem_size=N_FEAT,
                            single_packet=False,
                        )
                        colbase += ncols
                    for tau in range(TPB):
                        t = B * TPB + tau
                        if "no_pe" not in skip:
                            ps = psA.tile([P, P], F32, space="PSUM", tag="agg")
                        else:
                            ps = None
                        cols = []
                        for b in range(BANKS):
                            cb = TPB * int(np.sum(mb[:b])) + tau * int(mb[b])
                            cols.extend(range(cb, cb + int(mb[b])))
                        for ci, col in enumerate(cols):
                            oh = None
                            if "no_onehot" not in skip:
                                oh = wpool.tile([P, P], F16, tag="oh")
                                nc.vector.tensor_scalar(
                                    oh[:], iota_t[:], dst_t[:, col:col + 1],
                                    None, op0=mybir.AluOpType.is_equal)
                            if "no_pe" not in skip:
                                lhs = (gath[:, col * N_FEAT:(col + 1) * N_FEAT]
                                       if gath is not None else iota_t[:])
                                rhs = (oh[:] if "no_onehot" not in skip
                                       else ident_t[:])
                                nc.tensor.matmul(
                                    out=ps[:], lhsT=lhs, rhs=rhs,
                                    start=(ci == 0), stop=False)
                        # self-loop term: ps += g_own^T via identity matmul
                        gown = wpool.tile([P, N_FEAT], F16, tag="gown")
                        nc.sync.dma_start(gown[:], own[t * P:(t + 1) * P, :])
                        if "no_pe" not in skip:
                            nc.tensor.matmul(out=ps[:], lhsT=gown[:],
                                             rhs=ident_t[:], start=False,
                                             stop=True)
                        agg = wpool.tile([P, P], F16, tag="agg_sb")
                        nc.scalar.copy(agg[:],
                                       ps[:] if "no_pe" not in skip else iota_t[:])
                        if "no_pe" not in skip:
                            ph = psH.tile([P, P], F32, space="PSUM", tag="h")
                        else:
                            ph = None
                        if "no_pe" not in skip:
                            nc.tensor.matmul(out=ph[:], lhsT=agg[:],
                                             rhs=w_t[:], start=True, stop=True)
                        scale_ap = (dsqT_t if layer == 0 else disT_t)[:, t:t + 1]
                        hout = opool.tile([P, N_FEAT], F16, tag="gout")
                        nc.scalar.activation(hout[:],
                                             ph[:] if ph is not None else iota_t[:],
                                             mybir.ActivationFunctionType.Relu,
                                             scale=scale_ap)
                        if layer == 0:
                            nc.sync.dma_start(ag1_in[t * P:(t + 1) * P, :],
                                              hout[:])
                        else:
                            # tensor_tensor_reduce crashes HW; use 2 plain ops
                            scr = wpool.tile([P, P], F32, tag="scr")
                            nc.vector.tensor_tensor(
                                out=scr[:], in0=hout[:], in1=wfc_t[:],
                                op=mybir.AluOpType.mult)
                            nc.vector.tensor_reduce(
                                shared_t[:, t:t + 1], scr[:],
                                mybir.AxisListType.X, mybir.AluOpType.add)
                if layer == 0 and "no_ag" not in skip:
                    nc.gpsimd.collective_compute(
                        "AllGather", mybir.AluOpType.bypass, replica_groups=rg,
                        ins=[ag1_in[:]], outs=[g1_full[:]])
            nc.sync.dma_start(t_out[:], shared_t[:])

    nc.finalize()
    return nc


# ------------------------------------------------------------------- kernel
def kernel(x, edge_index, W1, b1, W2, b2, w_fc, th_bias):
    x = np.asarray(x)
    edge_index = np.asarray(edge_index)
    W1 = np.asarray(W1, np.float32)
    b1 = np.asarray(b1, np.float32)
    W2 = np.asarray(W2, np.float32)
    b2 = np.asarray(b2, np.float32)
    w_fc = np.asarray(w_fc, np.float32)
    th_bias = np.asarray(th_bias, np.float32)
    assert np.all(b1 == 0) and np.all(b2 == 0), "nonzero bias unsupported"

    dst = edge_index[1].astype(np.int64)
    deg = (np.bincount(dst, minlength=N_NODES) + 1.0).astype(np.float32)
    dis = 1.0 / np.sqrt(deg)

    ek = hash(edge_index.tobytes())
    if ek not in _CACHE:
        _CACHE.clear()
        _CACHE[ek] = _prep_edges(edge_index)
    prep = _CACHE[ek]
    K, colspb, mb = prep["K"], prep["colspb"], prep["mb"]

    pk = (K, tuple(mb))
    if ("prog", pk) not in _CACHE:
        _CACHE[("prog", pk)] = _build_program(
            K, colspb, prep["ni_b"], prep["call_off16"], mb, with_bias=False)
    nc = _CACHE[("prog", pk)]

    iota_np = np.broadcast_to(np.arange(P, dtype=np.float16), (P, P)).copy()
    ident_np = np.eye(P, dtype=np.float16)
    wfc_np = np.broadcast_to(w_fc[:, 0].astype(np.float16), (P, HIDDEN)).copy()
    w1h = W1.astype(np.float16)
    w2h = W2.astype(np.float16)

    in_maps = []
    for c in range(CORES):
        xs = np.zeros((SHARD_PAD, N_FEAT), np.float32)
        xs[:SHARD] = x[c * SHARD:(c + 1) * SHARD]
        d = np.zeros(SHARD_PAD, np.float32)
        d[:SHARD] = dis[c * SHARD:(c + 1) * SHARD]
        disT = d.reshape(TILES, P).T.copy()
        in_maps.append(dict(
            xs=xs, idx16=prep["idx16"][c], dstloc=prep["dstloc"][c],
            disT=disT, dissqT=(disT * disT), w1h=w1h, w2h=w2h,
            wfcb=wfc_np, iota=iota_np, ident=ident_np))

    res = run_bass_kernel_spmd(nc, in_maps, core_ids=list(range(CORES)))
    global LAST_EXEC_NS, LAST_TRACE
    LAST_EXEC_NS = res.exec_time_ns
    LAST_TRACE = res.instructions_and_trace

    shared = np.empty(N_NODES, np.float32)
    for c in range(CORES):
        o = res.results[c]["outs"]          # [128, TILES]
        shared[c * SHARD:(c + 1) * SHARD] = o.T.reshape(-1)[:SHARD]
    return shared[:, None] + th_bias[None, :]


if __name__ == "__main__":
    rng = np.random.default_rng(0)
    x = rng.normal(size=(N_NODES, N_FEAT)).astype(np.float32)
    ei = rng.integers(0, N_NODES, size=(2, 1600000)).astype(np.int64)
    s = 1.0 / np.sqrt(N_FEAT)
    W1 = rng.uniform(-s, s, size=(N_FEAT, HIDDEN)).astype(np.float32)
    W2 = rng.uniform(-s, s, size=(HIDDEN, HIDDEN)).astype(np.float32)
    w_fc = rng.uniform(-s, s, size=(HIDDEN, 1)).astype(np.float32)
    out = kernel(x=x, edge_index=ei, W1=W1, b1=np.zeros(HIDDEN, np.float32),
                 W2=W2, b2=np.zeros(HIDDEN, np.float32), w_fc=w_fc,
                 th_bias=np.zeros(NUM_CLASSES - 1, np.float32))
    print(out.shape, out.dtype)

